# revision 22
# baseline (speedup 1.0000x reference)
"""Trainium2 Bass kernel for nn_Diagomal_DWConv (diagonal depthwise conv).

Math (from the reference):
  View x as rows X[r, w], r in [0, R), R = B*C*H, W columns.
  out[r, w] = bias[c(r)] + sum_i weight[c(r), 0, i] * X[(r + 2 - i) mod R, w + i - 2]
  with zero padding in w only, c(r) = r // H mod C.

Strategy (one batch of 16384 rows per NeuronCore, all fp16 on the wire):
  - Rows live ON PARTITIONS, sheared so every tap of the diagonal conv hits
    the same free-dim column: partition p of block b holds row 128b-2+p with
    its columns shifted right by p relative to the block origin.  The shear
    is realized by a DMA whose SBUF partition stride is WI+1 halfwords: the
    DGE decomposes that as "+1 partition, +1 element", and the element
    offset accumulates mod 4, yielding a (p mod 4)-element offset; the
    4-aligned 4*(p//4) part of the shear is pre-baked into the DRAM slab
    by the host (which also bakes zero column padding).
  - One banded-stationary matmul per 128-row block computes all 5 taps in a
    single 379-column pass: stationary[k, m] = weight[c, m+4-k] (5
    diagonals), contraction over the 128 partitions/rows, producing 124
    output rows per block.  All 64 channel stationaries are precomputed on
    the host (2 MB) since each block spans exactly one channel.
  - The 4 leftover rows per block (m in [124,128)) are computed by a small
    rows-in-free "strip" pass: 20 diag-stationary matmuls, baseline-style.
  - PSUM mega-tiles of 4 banks hold 4 blocks; ScalarE and VectorE each
    drain half a group (f32 -> fp16 cast only; bias is added by the host).
  - The output leaves SHEARED as flat [128, 380]-per-block slabs (3 KB
    contiguous runs per partition); the host undoes the shear for free.
  - Input rides the SP HWDGE queue; stationaries + outputs ride ACT HWDGE.
"""

import numpy as np

import concourse.bacc as bacc
import concourse.tile as tile
import concourse.mybir as mybir
from concourse.bass_utils import run_bass_kernel_spmd

F16 = mybir.dt.float16
F32 = mybir.dt.float32

B, C, H, W = 8, 64, 256, 256
KS, PAD = 5, 2
R = B * C * H
NCORES = 8
RC = R // NCORES        # 16384 rows per core
NP = 128                # partitions
NBLK = RC // NP         # 128 blocks per core
M = 124                 # main-path output rows per block
SLOT = 268              # input row slot (2+256+2 zero pad + 3 residue + pad)
RUNLEN = NBLK * SLOT + 124  # per-partition full input run incl. group pad
SPLITS = [8, 24, 32, 32, 32]   # blocks per input split tile (progressive)
NSPLIT = len(SPLITS)
# per-split run: blocks*SLOT + 120 group-pad straddle; width pads the BIR bound
def _lns(nb):
    return nb * SLOT + 120
NS = 379                # matmul stream columns per block
OSLOT = 380             # sheared output slot per block (379 valid + 1)
OW = 4 * OSLOT          # output group tile width (1520)
OUTW = NBLK * OSLOT     # per-partition output run (48640)
SROWS, SW = 8, 260      # strip: 8 rows x 260 padded cols per partition
SINW = SROWS * SW       # 2080
NWARM = 40

_CACHE = {}


def _build_nc():
    nc = bacc.Bacc("TRN2", num_devices=NCORES)
    xs = nc.dram_tensor("xs", [sum(NP * _lns(nb) for nb in SPLITS)], F16, kind="ExternalInput")
    dall = nc.dram_tensor("dall", [NP * 64 * NP], F16, kind="ExternalInput")
    idm = nc.dram_tensor("idm", [NP * NP], F16, kind="ExternalInput")
    wst = nc.dram_tensor("wst", [NP * 8], F32, kind="ExternalInput")
    sin = nc.dram_tensor("sin", [NP * SINW], F16, kind="ExternalInput")
    yo = nc.dram_tensor("yo", [NP * OUTW], F16, kind="ExternalOutput")
    so = nc.dram_tensor("so", [NP * 1024], F16, kind="ExternalOutput")

    with tile.TileContext(nc) as tc:
        with (
            tc.tile_pool(name="const", bufs=1) as cpool,
            tc.tile_pool(name="outp", bufs=24) as opool,
            tc.tile_pool(name="ps", bufs=2, space="PSUM") as pspool,
        ):
            # --- PE warm-up (release HAM clock gate during the DMA head) ---
            wt = cpool.tile([NP, 256], F16)
            nc.gpsimd.memset(wt[:], 0.0)
            wps = pspool.tile([NP, 2048], F32, tag="ps")
            for _ in range(NWARM):
                nc.tensor.matmul(
                    wps[0:64, 0:256], wt[:, 0:64], wt[:, 0:256],
                    start=True, stop=True,
                )

            # --- input split tiles (deps stay per-split => overlap w/ PE) ---
            # garbage cols [0, p mod 4) are never read (streams start at col 4)
            mits = []
            offs = []
            xoff = 0
            for s, nb in enumerate(SPLITS):
                ln = _lns(nb)
                mits.append(cpool.tile([NP, ln + 128], F16, name=f"mit{s}"))
                offs.append(xoff)
                xoff += NP * ln

            def _load_split(s):
                ln = _lns(SPLITS[s])
                ws = ln + 128
                dst = mits[s][:, 0:ws].copy()
                dst.ap = mybir.VecI64Pair([[ws + 1, NP], [1, ln]])
                sap = xs.ap().copy()
                sap.ap = mybir.VecI64Pair([[ln, NP], [1, ln]])
                sap.offset = offs[s]
                nc.sync.dma_start(dst, sap)

            # banded stationaries: ONE issue on the parallel ACT ring (more
            # issues would head-of-line block the scalar drains behind them)
            dt_ = cpool.tile([NP, 64 * NP], F16)
            nc.scalar.dma_start(dt_[:], dall.ap())

            _load_split(0)
            _load_split(1)
            # strip input + constants land early (strip runs at block 67)
            sit = cpool.tile([NP, SINW], F16)
            nc.sync.dma_start(sit[:], sin.ap())
            idt = cpool.tile([NP, NP], F16)
            nc.sync.dma_start(idt[:], idm.ap())
            wstt = cpool.tile([NP, 8], F32)
            nc.sync.dma_start(wstt[:], wst.ap())
            for s in range(2, NSPLIT):
                _load_split(s)

            # --- strip diag stationaries (tap i scaled identity) ---
            dgt = cpool.tile([NP, KS * NP], F16)
            for i in range(KS):
                nc.vector.tensor_scalar(
                    dgt[:, i * NP : (i + 1) * NP], idt[:],
                    wstt[:, i : i + 1], None, mybir.AluOpType.mult,
                )

            # --- main block loop ---
            ps = None
            om = None
            for b in range(NBLK):
                j = b % 4
                if j == 0:
                    ps = pspool.tile([NP, 2048], F32, tag="ps")
                    om = opool.tile([NP, OW], F16)
                c = b >> 1
                sidx, lb, acc = 0, b, 0
                while lb >= SPLITS[sidx]:
                    lb -= SPLITS[sidx]
                    sidx += 1
                nc.tensor.matmul(
                    ps[:, j * 512 : j * 512 + NS],
                    dt_[:, c * NP : (c + 1) * NP],
                    mits[sidx][:, lb * SLOT + 4 : lb * SLOT + 4 + NS],
                    start=True, stop=True,
                )
                if j == 3:
                    # drain 4 blocks, split by columns (scalar ~1.5x slower)
                    SC = 188
                    ssrc = ps[:, 0:2048].copy()
                    ssrc.ap = mybir.VecI64Pair([[2048, NP], [512, 4], [1, SC]])
                    sdst = om[:, 0:OW].copy()
                    sdst.ap = mybir.VecI64Pair([[OW, NP], [OSLOT, 4], [1, SC]])
                    nc.scalar.activation(
                        sdst, ssrc, mybir.ActivationFunctionType.Copy
                    )
                    vsrc = ps[:, 0:2048].copy()
                    vsrc.ap = mybir.VecI64Pair([[2048, NP], [512, 4], [1, OSLOT - SC]])
                    vsrc.offset = SC
                    vdst = om[:, 0:OW].copy()
                    vdst.ap = mybir.VecI64Pair([[OW, NP], [OSLOT, 4], [1, OSLOT - SC]])
                    vdst.offset = SC
                    nc.vector.tensor_scalar_mul(vdst, vsrc, 1.0)
                    odst = yo.ap().copy()
                    odst.ap = mybir.VecI64Pair([[OUTW, NP], [1, OW]])
                    odst.offset = (b - 3) * OSLOT
                    nc.gpsimd.dma_start(odst, om[:])

                if b == 67:
                    # strip pass: 4 leftover rows per block, rows-in-free
                    sps = pspool.tile([NP, 2048], F32, tag="ps")
                    for t in range(4):
                        for i in range(KS):
                            rho = t + 4 - i
                            nc.tensor.matmul(
                                sps[:, t * 512 : t * 512 + W],
                                dgt[:, i * NP : (i + 1) * NP],
                                sit[:, rho * SW + i : rho * SW + i + W],
                                start=(i == 0), stop=(i == KS - 1),
                            )
                    sot = cpool.tile([NP, 1024], F16)
                    stsrc = sps[:, 0:1024].copy()
                    stsrc.ap = mybir.VecI64Pair([[2048, NP], [512, 4], [1, W]])
                    stdst = sot[:, 0:1024].copy()
                    stdst.ap = mybir.VecI64Pair([[1024, NP], [W, 4], [1, W]])
                    nc.scalar.activation(
                        stdst, stsrc, mybir.ActivationFunctionType.Copy
                    )
                    nc.gpsimd.dma_start(so.ap(), sot[:])

    nc.compile()
    return nc


def _host_prep(x, weight, bias):
    """Per-core in_maps: sheared input slab, strip slab, stationaries."""
    xr = np.ascontiguousarray(x, dtype=np.float32).reshape(R, W)
    x16 = xr.astype(np.float16)
    wgt = np.ascontiguousarray(weight, dtype=np.float32).reshape(C, KS)

    # banded stationaries: dall[k, c*128 + m] = w[c, m+4-k] on 5 diagonals
    # (columns m in [M, 128) stay zero: pads NumWeights to 128 to enable FWL)
    dall = np.zeros((NP, C * NP), dtype=np.float16)
    for dlt in range(KS):          # dlt = k - m in [0, 5)
        i = KS - 1 - dlt
        m = np.arange(M)
        for c in range(C):
            dall[m + dlt, c * NP + m] = np.float16(wgt[c, i])

    idv = np.eye(NP, dtype=np.float16)
    p = np.arange(NP)
    wstrip = np.zeros((NP, 8), dtype=np.float32)
    wstrip[:, 0:KS] = wgt[p >> 1]

    in_maps = []
    for k in range(NCORES):
        base = k * RC
        # extended local rows [-2, 16386) with global wrap
        qidx = (base + np.arange(-2, RC + 2)) % R
        xe = x16[qidx]                       # [RC+4, W]
        xp = np.zeros((RC + 4, SW), dtype=np.float16)
        xp[:, 2 : 2 + W] = xe                # padded cols [-2, 258)

        # main slab: slab[p, b*SLOT + 4*(p//4) + t] = xp[128b + p, t],
        # then cut into NSPLIT overlapping per-partition runs of LNS
        slab = np.zeros((NP, RUNLEN), dtype=np.float16)
        for pp in range(NP):
            gp = 4 * (pp >> 2)
            sv = slab[pp, gp : gp + NBLK * SLOT].reshape(NBLK, SLOT)
            sv[:, 0:SW] = xp[pp::NP][:NBLK]
        xsp = []
        b0 = 0
        for nb in SPLITS:
            xsp.append(slab[:, b0 * SLOT : b0 * SLOT + _lns(nb)].reshape(-1))
            b0 += nb
        xsplit = np.concatenate(xsp)

        # strip slab: partition p holds rows 128p+122..129 (xp idx = row+2)
        ridx = 128 * p[:, None] + 124 + np.arange(8)[None, :]  # [NP, 8]
        strip = xp[ridx.reshape(-1)].reshape(NP, SINW)

        in_maps.append({
            "xs": xsplit, "dall": dall.reshape(-1),
            "idm": idv.reshape(-1), "wst": wstrip.reshape(-1),
            "sin": strip.reshape(-1),
        })
    return in_maps


def kernel(x, weight, bias):
    x = np.asarray(x)
    weight = np.asarray(weight)
    bias = np.asarray(bias, dtype=np.float32)
    if "nc" not in _CACHE:
        _CACHE["nc"] = _build_nc()
    nc = _CACHE["nc"]
    in_maps = _host_prep(x, weight, bias)
    res = run_bass_kernel_spmd(nc, in_maps, list(range(NCORES)))

    out = np.empty((R, W), dtype=np.float32)
    for k in range(NCORES):
        yo = np.asarray(res.results[k]["yo"]).reshape(NP, NBLK, OSLOT)
        so = np.asarray(res.results[k]["so"]).reshape(NP, 4, W)
        yloc = out[k * RC : (k + 1) * RC].reshape(NBLK, NP, W)
        for m in range(M):
            yloc[:, m, :] = yo[m, :, m : m + W].astype(np.float32)
        yloc[:, M:NP, :] = so.astype(np.float32)
    out += np.tile(np.repeat(bias, H), B)[:, None]
    return out.reshape(B, C, H, W)


# revision 23
# speedup vs baseline: 1.0669x; 1.0669x over previous
"""Trainium2 Bass kernel for nn_Diagomal_DWConv (diagonal depthwise conv).

Math (from the reference):
  View x as rows X[r, w], r in [0, R), R = B*C*H, W columns.
  out[r, w] = bias[c(r)] + sum_i weight[c(r), 0, i] * X[(r + 2 - i) mod R, w + i - 2]
  with zero padding in w only, c(r) = r // H mod C.

Strategy (one batch of 16384 rows per NeuronCore, all fp16 on the wire):
  - Rows live ON PARTITIONS, sheared so every tap of the diagonal conv hits
    the same free-dim column: partition p of block b holds row 128b-2+p with
    its columns shifted right by p relative to the block origin.  The shear
    is realized by a DMA whose SBUF partition stride is WI+1 halfwords: the
    DGE decomposes that as "+1 partition, +1 element", and the element
    offset accumulates mod 4, yielding a (p mod 4)-element offset; the
    4-aligned 4*(p//4) part of the shear is pre-baked into the DRAM slab
    by the host (which also bakes zero column padding).
  - One banded-stationary matmul per 128-row block computes all 5 taps in a
    single 379-column pass: stationary[k, m] = weight[c, m+4-k] (5
    diagonals), contraction over the 128 partitions/rows, producing 124
    output rows per block.  All 64 channel stationaries are precomputed on
    the host (2 MB) since each block spans exactly one channel.
  - The 4 leftover rows per block (m in [124,128)) are computed by a small
    rows-in-free "strip" pass: 20 diag-stationary matmuls, baseline-style.
  - PSUM mega-tiles of 4 banks hold 4 blocks; ScalarE and VectorE each
    drain half a group (f32 -> fp16 cast only; bias is added by the host).
  - The output leaves SHEARED as flat [128, 380]-per-block slabs (3 KB
    contiguous runs per partition); the host undoes the shear for free.
  - Input rides the SP HWDGE queue; stationaries + outputs ride ACT HWDGE.
"""

import numpy as np

import concourse.bacc as bacc
import concourse.tile as tile
import concourse.mybir as mybir
from concourse.bass_utils import run_bass_kernel_spmd

F16 = mybir.dt.float16
F32 = mybir.dt.float32

B, C, H, W = 8, 64, 256, 256
KS, PAD = 5, 2
R = B * C * H
NCORES = 8
RC = R // NCORES        # 16384 rows per core
NP = 128                # partitions
NBLK = RC // NP         # 128 blocks per core
M = 124                 # main-path output rows per block
SLOT = 268              # input row slot (2+256+2 zero pad + 3 residue + pad)
RUNLEN = NBLK * SLOT + 124  # per-partition full input run incl. group pad
SPLITS = [8, 24, 32, 32, 32]   # blocks per input split tile (progressive)
NSPLIT = len(SPLITS)
# per-split run: blocks*SLOT + 120 group-pad straddle; width pads the BIR bound
def _lns(nb):
    return nb * SLOT + 120
NS = 379                # matmul stream columns per block
OSLOT = 380             # sheared output slot per block (379 valid + 1)
OW = 4 * OSLOT          # output group tile width (1520)
OUTW = NBLK * OSLOT     # per-partition output run (48640)
SROWS, SW = 8, 260      # strip: 8 rows x 260 padded cols per partition
SINW = SROWS * SW       # 2080
NWARM = 40

_CACHE = {}


def _build_nc():
    nc = bacc.Bacc("TRN2", num_devices=NCORES)
    xs = nc.dram_tensor("xs", [sum(NP * _lns(nb) for nb in SPLITS)], F16, kind="ExternalInput")
    dall = nc.dram_tensor("dall", [NP * 64 * NP], F16, kind="ExternalInput")
    idm = nc.dram_tensor("idm", [NP * NP], F16, kind="ExternalInput")
    wst = nc.dram_tensor("wst", [NP * 8], F32, kind="ExternalInput")
    sin = nc.dram_tensor("sin", [NP * SINW], F16, kind="ExternalInput")
    yo = nc.dram_tensor("yo", [NP * OUTW], F16, kind="ExternalOutput")
    so = nc.dram_tensor("so", [NP * 1024], F16, kind="ExternalOutput")

    with tile.TileContext(nc) as tc:
        with (
            tc.tile_pool(name="const", bufs=1) as cpool,
            tc.tile_pool(name="outp", bufs=24) as opool,
            tc.tile_pool(name="ps", bufs=2, space="PSUM") as pspool,
        ):
            # --- PE warm-up (release HAM clock gate during the DMA head) ---
            wt = cpool.tile([NP, 256], F16)
            nc.gpsimd.memset(wt[:], 0.0)
            wps = pspool.tile([NP, 2048], F32, tag="ps")
            for _ in range(NWARM):
                nc.tensor.matmul(
                    wps[0:64, 0:256], wt[:, 0:64], wt[:, 0:256],
                    start=True, stop=True,
                )

            # --- input split tiles (deps stay per-split => overlap w/ PE) ---
            # garbage cols [0, p mod 4) are never read (streams start at col 4)
            mits = []
            offs = []
            xoff = 0
            for s, nb in enumerate(SPLITS):
                ln = _lns(nb)
                mits.append(cpool.tile([NP, ln + 128], F16, name=f"mit{s}"))
                offs.append(xoff)
                xoff += NP * ln

            def _load_split(s):
                ln = _lns(SPLITS[s])
                ws = ln + 128
                dst = mits[s][:, 0:ws].copy()
                dst.ap = mybir.VecI64Pair([[ws + 1, NP], [1, ln]])
                sap = xs.ap().copy()
                sap.ap = mybir.VecI64Pair([[ln, NP], [1, ln]])
                sap.offset = offs[s]
                nc.sync.dma_start(dst, sap)

            # banded stationaries FIRST on the sync ring: one 2.1 MB DMA at
            # exclusive bandwidth so block 0 is never stationary-gated
            dt_ = cpool.tile([NP, 64 * NP], F16)
            nc.sync.dma_start(dt_[:], dall.ap())

            _load_split(0)
            _load_split(1)
            # strip input + constants land early (strip runs at block 67)
            sit = cpool.tile([NP, SINW], F16)
            nc.sync.dma_start(sit[:], sin.ap())
            idt = cpool.tile([NP, NP], F16)
            nc.sync.dma_start(idt[:], idm.ap())
            wstt = cpool.tile([NP, 8], F32)
            nc.sync.dma_start(wstt[:], wst.ap())
            for s in range(2, NSPLIT):
                _load_split(s)

            # --- strip diag stationaries (tap i scaled identity) ---
            dgt = cpool.tile([NP, KS * NP], F16)
            for i in range(KS):
                nc.vector.tensor_scalar(
                    dgt[:, i * NP : (i + 1) * NP], idt[:],
                    wstt[:, i : i + 1], None, mybir.AluOpType.mult,
                )

            # --- main block loop ---
            ps = None
            om = None
            for b in range(NBLK):
                j = b % 4
                if j == 0:
                    ps = pspool.tile([NP, 2048], F32, tag="ps")
                    om = opool.tile([NP, OW], F16)
                c = b >> 1
                sidx, lb, acc = 0, b, 0
                while lb >= SPLITS[sidx]:
                    lb -= SPLITS[sidx]
                    sidx += 1
                nc.tensor.matmul(
                    ps[:, j * 512 : j * 512 + NS],
                    dt_[:, c * NP : (c + 1) * NP],
                    mits[sidx][:, lb * SLOT + 4 : lb * SLOT + 4 + NS],
                    start=True, stop=True,
                )
                if j == 3:
                    # drain 4 blocks, split by columns (scalar ~1.5x slower)
                    SC = 188
                    ssrc = ps[:, 0:2048].copy()
                    ssrc.ap = mybir.VecI64Pair([[2048, NP], [512, 4], [1, SC]])
                    sdst = om[:, 0:OW].copy()
                    sdst.ap = mybir.VecI64Pair([[OW, NP], [OSLOT, 4], [1, SC]])
                    nc.scalar.activation(
                        sdst, ssrc, mybir.ActivationFunctionType.Copy
                    )
                    vsrc = ps[:, 0:2048].copy()
                    vsrc.ap = mybir.VecI64Pair([[2048, NP], [512, 4], [1, OSLOT - SC]])
                    vsrc.offset = SC
                    vdst = om[:, 0:OW].copy()
                    vdst.ap = mybir.VecI64Pair([[OW, NP], [OSLOT, 4], [1, OSLOT - SC]])
                    vdst.offset = SC
                    nc.vector.tensor_scalar_mul(vdst, vsrc, 1.0)
                    odst = yo.ap().copy()
                    odst.ap = mybir.VecI64Pair([[OUTW, NP], [1, OW]])
                    odst.offset = (b - 3) * OSLOT
                    nc.gpsimd.dma_start(odst, om[:])

                if b == 67:
                    # strip pass: 4 leftover rows per block, rows-in-free
                    sps = pspool.tile([NP, 2048], F32, tag="ps")
                    for t in range(4):
                        for i in range(KS):
                            rho = t + 4 - i
                            nc.tensor.matmul(
                                sps[:, t * 512 : t * 512 + W],
                                dgt[:, i * NP : (i + 1) * NP],
                                sit[:, rho * SW + i : rho * SW + i + W],
                                start=(i == 0), stop=(i == KS - 1),
                            )
                    sot = cpool.tile([NP, 1024], F16)
                    stsrc = sps[:, 0:1024].copy()
                    stsrc.ap = mybir.VecI64Pair([[2048, NP], [512, 4], [1, W]])
                    stdst = sot[:, 0:1024].copy()
                    stdst.ap = mybir.VecI64Pair([[1024, NP], [W, 4], [1, W]])
                    nc.scalar.activation(
                        stdst, stsrc, mybir.ActivationFunctionType.Copy
                    )
                    nc.gpsimd.dma_start(so.ap(), sot[:])

    nc.compile()
    return nc


def _host_prep(x, weight, bias):
    """Per-core in_maps: sheared input slab, strip slab, stationaries."""
    xr = np.ascontiguousarray(x, dtype=np.float32).reshape(R, W)
    x16 = xr.astype(np.float16)
    wgt = np.ascontiguousarray(weight, dtype=np.float32).reshape(C, KS)

    # banded stationaries: dall[k, c*128 + m] = w[c, m+4-k] on 5 diagonals
    # (columns m in [M, 128) stay zero: pads NumWeights to 128 to enable FWL)
    dall = np.zeros((NP, C * NP), dtype=np.float16)
    for dlt in range(KS):          # dlt = k - m in [0, 5)
        i = KS - 1 - dlt
        m = np.arange(M)
        for c in range(C):
            dall[m + dlt, c * NP + m] = np.float16(wgt[c, i])

    idv = np.eye(NP, dtype=np.float16)
    p = np.arange(NP)
    wstrip = np.zeros((NP, 8), dtype=np.float32)
    wstrip[:, 0:KS] = wgt[p >> 1]

    in_maps = []
    for k in range(NCORES):
        base = k * RC
        # extended local rows [-2, 16386) with global wrap
        qidx = (base + np.arange(-2, RC + 2)) % R
        xe = x16[qidx]                       # [RC+4, W]
        xp = np.zeros((RC + 4, SW), dtype=np.float16)
        xp[:, 2 : 2 + W] = xe                # padded cols [-2, 258)

        # main slab: slab[p, b*SLOT + 4*(p//4) + t] = xp[128b + p, t],
        # then cut into NSPLIT overlapping per-partition runs of LNS
        slab = np.zeros((NP, RUNLEN), dtype=np.float16)
        for pp in range(NP):
            gp = 4 * (pp >> 2)
            sv = slab[pp, gp : gp + NBLK * SLOT].reshape(NBLK, SLOT)
            sv[:, 0:SW] = xp[pp::NP][:NBLK]
        xsp = []
        b0 = 0
        for nb in SPLITS:
            xsp.append(slab[:, b0 * SLOT : b0 * SLOT + _lns(nb)].reshape(-1))
            b0 += nb
        xsplit = np.concatenate(xsp)

        # strip slab: partition p holds rows 128p+122..129 (xp idx = row+2)
        ridx = 128 * p[:, None] + 124 + np.arange(8)[None, :]  # [NP, 8]
        strip = xp[ridx.reshape(-1)].reshape(NP, SINW)

        in_maps.append({
            "xs": xsplit, "dall": dall.reshape(-1),
            "idm": idv.reshape(-1), "wst": wstrip.reshape(-1),
            "sin": strip.reshape(-1),
        })
    return in_maps


def kernel(x, weight, bias):
    x = np.asarray(x)
    weight = np.asarray(weight)
    bias = np.asarray(bias, dtype=np.float32)
    if "nc" not in _CACHE:
        _CACHE["nc"] = _build_nc()
    nc = _CACHE["nc"]
    in_maps = _host_prep(x, weight, bias)
    res = run_bass_kernel_spmd(nc, in_maps, list(range(NCORES)))

    out = np.empty((R, W), dtype=np.float32)
    for k in range(NCORES):
        yo = np.asarray(res.results[k]["yo"]).reshape(NP, NBLK, OSLOT)
        so = np.asarray(res.results[k]["so"]).reshape(NP, 4, W)
        yloc = out[k * RC : (k + 1) * RC].reshape(NBLK, NP, W)
        for m in range(M):
            yloc[:, m, :] = yo[m, :, m : m + W].astype(np.float32)
        yloc[:, M:NP, :] = so.astype(np.float32)
    out += np.tile(np.repeat(bias, H), B)[:, None]
    return out.reshape(B, C, H, W)
